# revision 4
# baseline (speedup 1.0000x reference)
"""AverageNode2Vec on 8 Trainium2 NeuronCores - v3 (bounds-skip gathers).

Measured (serialized K-loop slope, full vocab, all 8 cores):
  37.2 us/iteration vs v2's 54.4 us pipelined / 73.9 us graded single-shot.
  Graded single-shot estimate ~41 us (slope + idx lead-in + out DMA).

Why it is faster than v2:
 - v2 gathered a padded row for every slack slot in its width-compacted
   schedule (57K rows/core at ~14.8 ns/row/engine = 52.7 us drain). v3
   points every padding slot at an out-of-bounds sentinel and passes
   bounds_check + oob_is_err=False: the SWDGE skips those entries, so only
   the ~39.5K real rows/core cost descriptors -> 36.5 us drain floor,
   measured 37.2 us. Skipped slots write nothing; the staging holes read
   as zeros (SBUF NEFF-load state, stable across executions since the
   holes are never written - verified on HW across processes/executions).
 - Globally-uniform round widths (max over the 8 cores): same schedule on
   every core -> no per-core Switch bodies, no Fori, a ~200-instruction
   static program; the +7% stream padding is free because pads are skipped.
 - G=8 column groups (group = t): gathers stream group-by-group; DVE
   folds + dot-products + ACT softplus chain of group g overlap gather
   g+1, so the single-shot tail is one group's epilogue, not all 56 cols.
 - Round 0 gathers straight into the accumulator region (len >= 1 always),
   saving one full fold pass.
 - Table bf16: fp8 halves bytes but not descriptor cost (drain is
   per-descriptor-bound), and fp8 DVE folds are slower than bf16 - fp8
   measured 86 us vs bf16 63.7 us on a bank-conflicted small table, and
   fp8's DVE fold volume alone exceeds the full-vocab drain floor.

HW contract notes (inherited + new):
 - indirect_dma_start: dst [128, n*D]; idx [128, n] int32;
   dst[p, j*D:(j+1)*D] = table[idx[p, j]]; entries > bounds_check are
   silently skipped with oob_is_err=False (no descriptor cost).
 - CCE compute_op on the indirect path is device-fatal; fold on DVE.
 - value_load()-style SEQ_ASSERT is not encodable by this walrus build;
   use reg_load + snap(min_val, max_val) (see vload()).
 - DVE tensor_tensor with fp8 inputs and bf16 output is device-fatal;
   keep DVE op dtypes uniform (bf16 everywhere here).
 - Broadcast APs (unsqueeze + broadcast_to) work on DVE tensor_tensor.

Flags (env, defaults are the graded config):
  KV3_DTYPE = bf16 | fp8, KV3_SKIP = 1|0, KV3_GROUPS = 8|4|2|1,
  KV3_REPEAT = 1 (graded) | 0 (runtime-K serialized bench, KV3_K),
  KV3_VOCAB = table-size override for cheap benching (bench only).
"""
import os
import numpy as np

VOCAB = 1_000_000
D = 128
B = 8192
NEG = 5
L = 10
NCORES = 8
T = 8                       # groups per core (group = t), 128 rows each
NTY = 7                     # u, v, n0..n4
SENT = 2 * VOCAB + 5        # OOB sentinel (> bounds_check)

_STATE = {}
LAST_EXEC_NS = None


GP = int(os.environ.get("KV3_GROUPS", "8"))     # gather groups
TPG = T // GP                                    # t's per group


def _schedule(lu_o, lv_o, ln_o):
    """Global (core-uniform) column order + widths per super-group.

    Super-group G = t in [G*TPG, (G+1)*TPG): NC=7*TPG columns, labeled
    ti*7+type (ti = t within group). Columns sorted by global maxlen.
    """
    chunks_l = np.stack([lu_o, lv_o] + [ln_o[:, n] for n in range(NEG)],
                        axis=1).reshape(64, 128, NTY)
    ncol = NTY * TPG
    scheds = []
    for G in range(GP):
        gmax = np.zeros(ncol, np.int64)
        for ti in range(TPG):
            t = G * TPG + ti
            for c in range(NCORES):
                gmax[ti * NTY:(ti + 1) * NTY] = np.maximum(
                    gmax[ti * NTY:(ti + 1) * NTY],
                    chunks_l[8 * t + c].max(axis=0))
        order = np.lexsort((np.arange(ncol), -gmax))
        widths = [int((gmax[order] > r).sum()) for r in range(L)]
        assert widths[0] == ncol
        pos_of = np.empty(ncol, np.int64)
        pos_of[order] = np.arange(ncol)
        scheds.append({"order": order, "widths": widths,
                       "pos_of": pos_of, "gmax": gmax[order]})
    return scheds


def _build_program(scheds, repeat=1, tdt_name="fp8", skip=True, vocab=VOCAB):
    import concourse.bass as bass
    import concourse.mybir as mybir
    from concourse.bass import IndirectOffsetOnAxis

    f32, i32 = mybir.dt.float32, mybir.dt.int32
    tdt = {"bf16": mybir.dt.bfloat16, "fp8": mybir.dt.float8e4}[tdt_name]
    pdt = tdt                          # product dtype (same-dtype DVE path)

    widths = [s["widths"] for s in scheds]
    S_g = [sum(w) for w in widths]              # stream cols per group
    STREAM = sum(S_g)
    # group g region: [acc: 7 cols][stage: rounds 1..9]
    goff = np.cumsum([0] + S_g)[:-1]            # stream/buffer col offset

    nc = bass.Bass()
    big = nc.dram_tensor("big", [2 * vocab + 8, D], tdt, kind="ExternalInput")
    idx_in = nc.dram_tensor("idx", [128, STREAM], i32, kind="ExternalInput")
    ruv_in = nc.dram_tensor("ruv", [128, T], f32, kind="ExternalInput")
    rn_in = nc.dram_tensor("rn", [128, T * NEG], f32, kind="ExternalInput")
    out = nc.dram_tensor("lp", [128, 2 * T], f32, kind="ExternalOutput")
    kv_in = None
    if repeat == 0:
        kv_in = nc.dram_tensor("kv", [1, 2], i32, kind="ExternalInput")

    def vload(eng, ap, lo=1, hi=1 << 22):
        tmp = eng.alloc_register(f"vl_{nc.next_id()}")
        eng.reg_load(tmp, ap)
        return eng.snap(tmp, donate=True, min_val=lo, max_val=hi)

    from contextlib import ExitStack
    ctx = ExitStack()
    with ctx:
        idx_t = ctx.enter_context(nc.sbuf_tensor([128, STREAM], i32))
        buf = ctx.enter_context(nc.sbuf_tensor([128, STREAM * D], tdt))
        ruv_t = ctx.enter_context(nc.sbuf_tensor([128, T], f32))
        rn_t = ctx.enter_context(nc.sbuf_tensor([128, T * NEG], f32))
        prod = ctx.enter_context(nc.sbuf_tensor([128, T * 6 * D], pdt))
        sraw = ctx.enter_context(nc.sbuf_tensor([128, T], f32))
        nraw = ctx.enter_context(nc.sbuf_tensor([128, T * NEG], f32))
        score = ctx.enter_context(nc.sbuf_tensor([128, T], f32))
        nscore = ctx.enter_context(nc.sbuf_tensor([128, T * NEG], f32))
        plt_t = ctx.enter_context(nc.sbuf_tensor([128, T], f32))
        nlt_t = ctx.enter_context(nc.sbuf_tensor([128, T * NEG], f32))
        lp_t = ctx.enter_context(nc.sbuf_tensor([128, 2 * T], f32))
        kv_t = None
        if repeat == 0:
            kv_t = ctx.enter_context(nc.sbuf_tensor([1, 2], i32))
        s_idx = ctx.enter_context(nc.semaphore("s_idx"))
        s_rcp = ctx.enter_context(nc.semaphore("s_rcp"))
        s_g = ctx.enter_context(nc.semaphore("s_g"))
        s_dve = ctx.enter_context(nc.semaphore("s_dve"))
        s_act = ctx.enter_context(nc.semaphore("s_act"))
        s_out = ctx.enter_context(nc.semaphore("s_out"))
        block = ctx.enter_context(nc.Block())

        bc = dict(bounds_check=2 * vocab + 1, oob_is_err=False) if skip else {}

        @block.sync
        def _(sync):
            # idx chunk 0 first: it gates the first gather. Recips are only
            # read by the DVE epilogue several us later.
            sync.dma_start(
                out=idx_t[:, goff[0]:goff[0] + S_g[0]],
                in_=idx_in[:, goff[0]:goff[0] + S_g[0]],
            ).then_inc(s_idx, 16)
            sync.dma_start(out=ruv_t[:], in_=ruv_in[:]).then_inc(s_rcp, 16)
            sync.dma_start(out=rn_t[:], in_=rn_in[:]).then_inc(s_rcp, 16)
            if repeat == 0:
                sync.dma_start(out=kv_t[:], in_=kv_in[:]).then_inc(s_rcp, 16)
            for g in range(1, GP):
                sync.dma_start(
                    out=idx_t[:, goff[g]:goff[g] + S_g[g]],
                    in_=idx_in[:, goff[g]:goff[g] + S_g[g]],
                ).then_inc(s_idx, 16)
            # split output: first half's fixed DMA cost hides under the
            # last groups' compute
            sync.wait_ge(s_act, T)
            sync.dma_start(out=out[:, 0:T], in_=lp_t[:, 0:T]).then_inc(
                s_out, 16)
            sync.wait_ge(s_act, 2 * T)
            sync.dma_start(out=out[:, T:2 * T],
                           in_=lp_t[:, T:2 * T]).then_inc(s_out, 16)

        @block.gpsimd
        def _(gpsimd):
            if repeat == 0:
                gpsimd.wait_ge(s_rcp, 48)
                krep = vload(gpsimd, kv_t[0:1, 0:1])

            def gather_iter(it=None):
                tgt = None
                if it is not None:
                    # serialize iterations: wait prior iter's ACT done
                    tgt = gpsimd.snap(gpsimd.to_reg(it * 2 * T), donate=True)
                for g in range(GP):
                    if it is None:
                        gpsimd.wait_ge(s_idx, 16 * (g + 1))
                    elif g == 0:
                        gpsimd.wait_ge(s_act, tgt)
                    gpsimd.indirect_dma_start(
                        out=buf[:, goff[g] * D:(goff[g] + S_g[g]) * D],
                        out_offset=None,
                        in_=big[:],
                        in_offset=IndirectOffsetOnAxis(
                            ap=idx_t[:, goff[g]:goff[g] + S_g[g]], axis=0),
                        **bc,
                    ).then_inc(s_g, 16)

            if repeat == 0:
                with gpsimd.Fori(0, krep) as it:
                    gather_iter(it)
            else:
                gather_iter()

        @block.vector
        def _(vector):
            vector.wait_ge(s_rcp, 48 if repeat == 0 else 32)

            def dve_group(g, base):
                s = scheds[g]
                w = s["widths"]
                ncol = NTY * TPG
                acc0 = goff[g] * D
                vector.wait_ge(s_g, base + 16 * (g + 1))
                off = goff[g] + ncol
                for r in range(1, L):
                    if w[r] == 0:
                        break
                    vector.tensor_tensor(
                        out=buf[:, acc0:acc0 + w[r] * D],
                        in0=buf[:, acc0:acc0 + w[r] * D],
                        in1=buf[:, off * D:(off + w[r]) * D],
                        op=mybir.AluOpType.add,
                    )
                    off += w[r]
                for ti in range(TPG):
                    t = g * TPG + ti
                    pu = int(s["pos_of"][ti * NTY + 0])
                    pv = int(s["pos_of"][ti * NTY + 1])
                    pn = [int(s["pos_of"][ti * NTY + 2 + n])
                          for n in range(NEG)]
                    po = t * 6 * D
                    vector.tensor_tensor(
                        out=prod[:, po:po + D],
                        in0=buf[:, acc0 + pu * D:acc0 + (pu + 1) * D],
                        in1=buf[:, acc0 + pv * D:acc0 + (pv + 1) * D],
                        op=mybir.AluOpType.mult,
                    )
                    n = 0
                    while n < NEG:
                        m = n
                        while m + 1 < NEG and pn[m + 1] == pn[m] + 1:
                            m += 1
                        cnt = m - n + 1
                        u_ap = buf[:, acc0 + pu * D:acc0 + (pu + 1) * D]
                        if cnt > 1:
                            u_ap = u_ap.unsqueeze(1).broadcast_to(
                                [128, cnt, D])
                            o_ap = prod[:, po + (1 + n) * D:
                                        po + (1 + m + 1) * D]\
                                .rearrange("p (t d) -> p t d", d=D)
                            i_ap = buf[:, acc0 + pn[n] * D:
                                       acc0 + (pn[m] + 1) * D]\
                                .rearrange("p (t d) -> p t d", d=D)
                        else:
                            o_ap = prod[:, po + (1 + n) * D:
                                        po + (1 + m + 1) * D]
                            i_ap = buf[:, acc0 + pn[n] * D:
                                       acc0 + (pn[m] + 1) * D]
                        vector.tensor_tensor(
                            out=o_ap, in0=i_ap, in1=u_ap,
                            op=mybir.AluOpType.mult,
                        )
                        n = m + 1
                    vector.tensor_reduce(
                        out=sraw[:, t:t + 1],
                        in_=prod[:, po:po + D].rearrange(
                            "p (t d) -> p t d", d=D),
                        axis=mybir.AxisListType.X,
                        op=mybir.AluOpType.add,
                    )
                    vector.tensor_reduce(
                        out=nraw[:, t * NEG:(t + 1) * NEG],
                        in_=prod[:, po + D:po + 6 * D].rearrange(
                            "p (t d) -> p t d", d=D),
                        axis=mybir.AxisListType.X,
                        op=mybir.AluOpType.add,
                    )
                    vector.tensor_tensor(
                        out=score[:, t:t + 1], in0=sraw[:, t:t + 1],
                        in1=ruv_t[:, t:t + 1], op=mybir.AluOpType.mult,
                    )
                    tt = vector.tensor_tensor(
                        out=nscore[:, t * NEG:(t + 1) * NEG],
                        in0=nraw[:, t * NEG:(t + 1) * NEG],
                        in1=rn_t[:, t * NEG:(t + 1) * NEG],
                        op=mybir.AluOpType.mult,
                    )
                    tt.then_inc(s_dve, 1)

            if repeat == 0:
                krep = vload(vector, kv_t[0:1, 0:1])
                with vector.Fori(0, krep) as it:
                    base = vector.snap(vector.to_reg(it * 16 * GP),
                                       donate=True)
                    for g in range(GP):
                        dve_group(g, base)
            else:
                for g in range(GP):
                    dve_group(g, 0)

        @block.scalar
        def _(scalar):
            def act_group(g, base):
                scalar.wait_ge(s_dve, base + g + 1)  # s_dve incs once per t
                scalar.activation(
                    out=plt_t[:, g:g + 1], in_=score[:, g:g + 1],
                    func=mybir.ActivationFunctionType.Exp, scale=-1.0,
                )
                scalar.activation(
                    out=plt_t[:, g:g + 1], in_=plt_t[:, g:g + 1],
                    func=mybir.ActivationFunctionType.Ln,
                    bias=1.0, accum_out=lp_t[:, 2 * g:2 * g + 1],
                ).then_inc(s_act, 1)
                scalar.activation(
                    out=nlt_t[:, g * NEG:(g + 1) * NEG],
                    in_=nscore[:, g * NEG:(g + 1) * NEG],
                    func=mybir.ActivationFunctionType.Exp, scale=1.0,
                )
                scalar.activation(
                    out=nlt_t[:, g * NEG:(g + 1) * NEG],
                    in_=nlt_t[:, g * NEG:(g + 1) * NEG],
                    func=mybir.ActivationFunctionType.Ln,
                    bias=1.0, accum_out=lp_t[:, 2 * g + 1:2 * g + 2],
                ).then_inc(s_act, 1)

            if repeat == 0:
                scalar.wait_ge(s_rcp, 48)
                krep = vload(scalar, kv_t[0:1, 0:1])
                with scalar.Fori(0, krep) as it:
                    base = scalar.snap(scalar.to_reg(it * T), donate=True)
                    for g in range(T):
                        act_group(g, base)
            else:
                for g in range(T):
                    act_group(g, 0)

    return nc


def _prep(pos_u, pos_u_lens, pos_v, pos_v_lens, neg_v, neg_v_lens,
          scale2, skip):
    lu = pos_u_lens.astype(np.int64)
    lv = pos_v_lens.astype(np.int64)
    ln = neg_v_lens.reshape(B, NEG).astype(np.int64)
    narg = np.argsort(-ln, axis=1, kind="stable")
    ln_s = np.take_along_axis(ln, narg, axis=1)
    nv_s = np.take_along_axis(neg_v.reshape(B, NEG, L), narg[:, :, None],
                              axis=1)

    key = np.lexsort((-(lu + lv), -ln_s[:, 4], -ln_s[:, 3], -ln_s[:, 2],
                      -ln_s[:, 1], -ln_s[:, 0]))
    lu_o, lv_o, ln_o = lu[key], lv[key], ln_s[key]
    pu_o, pv_o, nv_o = pos_u[key], pos_v[key], nv_s[key]

    scheds = _schedule(lu_o, lv_o, ln_o)

    # per-type padded index matrix per row: [B, NTY, L]
    # u -> row; v/neg -> row + VOCAB; pad -> SENT (skip) or 0
    pad = SENT if skip else 0
    idx_full = np.full((B, NTY, L), pad, np.int64)
    ar = np.arange(L)[None, :]
    mu = ar < lu_o[:, None]
    idx_full[:, 0, :] = np.where(mu, pu_o, pad)
    mv = ar < lv_o[:, None]
    idx_full[:, 1, :] = np.where(mv, pv_o + VOCAB, pad)
    for n in range(NEG):
        mn = ar < ln_o[:, n][:, None]
        idx_full[:, 2 + n, :] = np.where(mn, nv_o[:, n, :] + VOCAB, pad)

    chunks_i = idx_full.reshape(64, 128, NTY, L)
    lu_c = lu_o.reshape(64, 128)
    lv_c = lv_o.reshape(64, 128)
    ln_c = ln_o.reshape(64, 128, NEG)

    metas = []
    for c in range(NCORES):
        streams = []
        ruv = np.zeros((128, T), np.float64)
        rn = np.zeros((128, T * NEG), np.float64)
        for G in range(GP):
            s = scheds[G]
            # columns ti*7+type, stacked over ti then ordered by s["order"]
            idx_G = np.concatenate(
                [chunks_i[8 * (G * TPG + ti) + c] for ti in range(TPG)],
                axis=1)                                # [128, 7*TPG, L]
            idx_G = idx_G[:, s["order"], :]
            cols = [idx_G[:, :, 0]]
            for r in range(1, L):
                w = s["widths"][r]
                if w == 0:
                    break
                cols.append(idx_G[:, :w, r])
            streams.append(np.concatenate(cols, axis=1))
            for ti in range(TPG):
                t = G * TPG + ti
                k = 8 * t + c
                ruv[:, t] = scale2 / (lu_c[k] * lv_c[k])
                for n in range(NEG):
                    rn[:, t * NEG + n] = scale2 / (lu_c[k] * ln_c[k][:, n])
        idx = np.concatenate(streams, axis=1).astype(np.int32)
        metas.append({"idx": np.ascontiguousarray(idx),
                      "ruv": ruv.astype(np.float32),
                      "rn": np.ascontiguousarray(rn.astype(np.float32))})
    return scheds, metas


def kernel(u_table, v_table, pos_u, pos_u_lens, pos_v, pos_v_lens,
           neg_v, neg_v_lens):
    global LAST_EXEC_NS
    from concourse.bass_utils import run_bass_kernel_spmd

    u_table = np.asarray(u_table)
    v_table = np.asarray(v_table)
    pos_u = np.asarray(pos_u)
    pos_v = np.asarray(pos_v)
    neg_v = np.asarray(neg_v)
    pos_u_lens = np.asarray(pos_u_lens)
    pos_v_lens = np.asarray(pos_v_lens)
    neg_v_lens = np.asarray(neg_v_lens)

    tdt_name = os.environ.get("KV3_DTYPE", "bf16")
    skip = os.environ.get("KV3_SKIP", "1") == "1"
    repeat = int(os.environ.get("KV3_REPEAT", "1"))

    big = np.zeros((2 * VOCAB + 8, D), np.float32)
    big[:VOCAB] = u_table
    big[VOCAB:2 * VOCAB] = v_table
    big[0] = 0.0
    big[VOCAB] = 0.0
    import ml_dtypes
    if tdt_name == "fp8":
        scale2 = 1.0 / (256.0 * 256.0)
        big = (big * 256.0).astype(ml_dtypes.float8_e4m3)
    else:
        scale2 = 1.0
        big = big.astype(ml_dtypes.bfloat16)

    scheds, metas = _prep(pos_u, pos_u_lens, pos_v, pos_v_lens,
                          neg_v, neg_v_lens, scale2, skip)

    vocab = int(os.environ.get("KV3_VOCAB", str(VOCAB)))
    if vocab != VOCAB:
        # bench-only: shrink table, remap idx (same descriptor counts)
        big = np.ascontiguousarray(
            np.concatenate([big[:vocab], big[VOCAB:VOCAB + vocab],
                            big[-8:]], axis=0))
        for m in metas:
            ix = m["idx"].astype(np.int64)
            sent = ix >= 2 * VOCAB
            isv = (ix >= VOCAB) & ~sent
            ixn = np.where(sent, 2 * vocab + 5,
                           np.where(isv, vocab + (ix - VOCAB) % vocab,
                                    ix % vocab))
            m["idx"] = ixn.astype(np.int32)

    pkey = (tuple(tuple(s["widths"]) for s in scheds), tdt_name, skip,
            repeat, vocab, GP)
    if pkey not in _STATE:
        _STATE[pkey] = _build_program(scheds, repeat=repeat,
                                      tdt_name=tdt_name, skip=skip,
                                      vocab=vocab)
    nc = _STATE[pkey]

    in_maps = [{"big": big, **m} for m in metas]
    if repeat == 0:
        k = int(os.environ.get("KV3_K", "64"))
        for m in in_maps:
            m["kv"] = np.array([[k, 0]], np.int32)
    res = run_bass_kernel_spmd(nc, in_maps, core_ids=list(range(NCORES)))
    LAST_EXEC_NS = res.exec_time_ns

    total = np.float64(0.0)
    for r in res.results:
        total += r["lp"].astype(np.float64).sum()
    return np.float32(total / B)
